# revision 56
# baseline (speedup 1.0000x reference)
"""Trainium2 Bass kernel for a dense cross-task transformer block.

Math notes
----------
The reference "attention" has sequence length 1 on the key axis, so
softmax(scores) == 1.0 exactly and the whole q/k/score path is dead:

    mha_len1(q_in, kv_in, ...) == (kv_in @ wv.T + bv) @ wo.T + bo

which folds (on host) into a single matmul with W = wo @ wv and
b = wo @ bv + bo.  The block is then:

    verb1 = LN(verb + noun @ W1.T + c1)          (ln_v)
    verb2 = verb1 + FFN_v(verb1)
    noun1 = LN(noun + verb2 @ W2.T + c2)         (ln_n)
    noun2 = noun1 + FFN_n(noun1)
    return verb2, noun2

The LN shift beta folds into the FFN biases on host (b1' = b1 + w1@beta,
b2' = b2 + beta), so the device only applies (x - mean) * rstd * gamma.

Device strategy
---------------
Pure data parallel over 8 cores (batch 16384 -> 2048 rows/core), weights
replicated.  Feature-major layout ([E, batch]); all matmul operands are
fp16 (fp32 PSUM accumulation, fp32 LN statistics).

Single fused pipeline over 4 column chunks of 512: per chunk, stage A
(verb<-noun attn + LN), B (verb FFN), C (noun<-verb2 attn + LN, verb2
straight from SBUF), D (noun FFN).  Emission is software-pipelined so
the in-order PE never idles on a LayerNorm tail: D(c-1) covers A(c)'s
tail, and A(c+1)'s main matmuls are hoisted before C(c)'s tail.
LN stats use ones-vector matmuls (lag-2 interleaved with the mains);
rstd = Exp(-0.5*Ln(var+eps)) on ScalarE (both in one ACT table set, and
a dummy Ln after each gelu batch prefetches the set switch off the
critical path).  Outputs are fp16, upcast on host.
"""

import numpy as np
from contextlib import ExitStack

import concourse.bass as bass
import concourse.bass_isa as bass_isa
import concourse.bacc as bacc_mod
import concourse.mybir as mybir
import concourse.tile as tile
from concourse.bass_utils import run_bass_kernel_spmd

E = 1024          # embed dim
H2 = 2048         # FFN hidden dim
B_TOTAL = 16384
NCORES = 8
B = B_TOTAL // NCORES   # 2048 rows per core
P = 128
EPS = 1e-5
CHUNK = 512
NCH = B // CHUNK  # 4
KT = E // P       # 8
MT = E // P       # 8
HT = H2 // P      # 16

F32 = mybir.dt.float32
F16 = mybir.dt.float16
AF = mybir.ActivationFunctionType
OP = mybir.AluOpType


def _load_pvec(nc, pool, dram_ap, ntiles, tag):
    """DRAM [128, ntiles] (host-packed, contiguous) -> SBUF [128, ntiles]."""
    t = pool.tile([P, ntiles], F32, tag=tag, name=tag)
    nc.sync.dma_start(out=t[:], in_=dram_ap[:, :])
    return t


def _pack_pvec(v):
    """[ntiles*128] -> [128, ntiles] with element (p,t) = v[t*128+p]."""
    return np.ascontiguousarray(np.asarray(v, np.float32).reshape(-1, P).T)


def _build_program():
    nc = bacc_mod.Bacc("TRN2", target_bir_lowering=False)

    vT = nc.declare_dram_parameter("vT", [E, B], F16, isOutput=False)
    nT = nc.declare_dram_parameter("nT", [E, B], F16, isOutput=False)
    wvo1 = nc.declare_dram_parameter("wvo1", [E, E], F16, isOutput=False)   # (wo@wv).T : [k, m]
    bvo1 = nc.declare_dram_parameter("bvo1", [P, MT], F32, isOutput=False)
    wvo2 = nc.declare_dram_parameter("wvo2", [E, E], F16, isOutput=False)
    bvo2 = nc.declare_dram_parameter("bvo2", [P, MT], F32, isOutput=False)
    lnvg = nc.declare_dram_parameter("lnvg", [P, MT], F32, isOutput=False)
    lnng = nc.declare_dram_parameter("lnng", [P, MT], F32, isOutput=False)
    w1v = nc.declare_dram_parameter("w1v", [E, H2], F16, isOutput=False)    # fv_w1.T
    b1v = nc.declare_dram_parameter("b1v", [P, HT], F32, isOutput=False)
    w2v = nc.declare_dram_parameter("w2v", [H2, E], F16, isOutput=False)    # fv_w2.T
    b2v = nc.declare_dram_parameter("b2v", [P, MT], F32, isOutput=False)
    w1n = nc.declare_dram_parameter("w1n", [E, H2], F16, isOutput=False)
    b1n = nc.declare_dram_parameter("b1n", [P, HT], F32, isOutput=False)
    w2n = nc.declare_dram_parameter("w2n", [H2, E], F16, isOutput=False)
    b2n = nc.declare_dram_parameter("b2n", [P, MT], F32, isOutput=False)
    verb_out = nc.declare_dram_parameter("verb_out", [E, B], F16, isOutput=True)
    noun_out = nc.declare_dram_parameter("noun_out", [E, B], F16, isOutput=True)
    scratch = nc.declare_dram_parameter("scratch", [1, 1], F32, isOutput=False)

    with tile.TileContext(nc) as tc, ExitStack() as ctx:
        const = ctx.enter_context(tc.tile_pool(name="const", bufs=1))
        wvp = ctx.enter_context(tc.tile_pool(name="wvp", bufs=2))
        w1p = ctx.enter_context(tc.tile_pool(name="w1p", bufs=1))
        w2p = ctx.enter_context(tc.tile_pool(name="w2p", bufs=1))
        nounp = ctx.enter_context(tc.tile_pool(name="nounp", bufs=2))
        vresp = ctx.enter_context(tc.tile_pool(name="vresp", bufs=1))
        xp = ctx.enter_context(tc.tile_pool(name="xp", bufs=2))
        sqp = ctx.enter_context(tc.tile_pool(name="sqp", bufs=3))
        y1p = ctx.enter_context(tc.tile_pool(name="y1p", bufs=1))
        hp = ctx.enter_context(tc.tile_pool(name="hp", bufs=1))
        vo_p = ctx.enter_context(tc.tile_pool(name="vo_p", bufs=1))
        no_p = ctx.enter_context(tc.tile_pool(name="no_p", bufs=3))
        smp = ctx.enter_context(tc.tile_pool(name="smp", bufs=1))
        stp = ctx.enter_context(tc.tile_pool(name="stp", bufs=2))
        arp = ctx.enter_context(tc.tile_pool(name="arp", bufs=1))
        bbp = ctx.enter_context(tc.tile_pool(name="bbp", bufs=2))
        # PSUM pools (stats/broadcast banks freed by the GPSIMD all-reduce
        # go to deeper main-matmul pipelining)
        mps = ctx.enter_context(tc.tile_pool(name="mps", bufs=5, space="PSUM"))
        wup = ctx.enter_context(tc.tile_pool(name="wup", bufs=1, space="PSUM"))

        # ---- PE warmup: dense matmuls with no DMA deps, trips HAM to 8/8
        warm_w = const.tile([P, P], F16, tag="warm_w", name="warm_w")
        nc.vector.memset(warm_w[:], 1.0)
        warm_r = const.tile([P, 256], F16, tag="warm_r", name="warm_r")
        nc.vector.memset(warm_r[:], 0.0)
        wps = wup.tile([P, 256], F32, tag="wps", name="wps")
        for i in range(40):
            nc.tensor.matmul(wps[:], lhsT=warm_w[:], rhs=warm_r[:],
                             start=(i == 0), stop=(i == 39))

        ones_col = const.tile([P, 1], F16, tag="ones_col", name="ones_col")
        nc.vector.memset(ones_col[:], 1.0)
        ones_row = const.tile([1, P], F16, tag="ones_row", name="ones_row")
        nc.vector.memset(ones_row[:], 1.0)
        ones_row_f = const.tile([1, P], F32, tag="ones_row_f", name="ones_row_f")
        nc.vector.memset(ones_row_f[:], 1.0)
        eps_t = const.tile([1, 1], F32, tag="eps", name="eps")
        nc.vector.memset(eps_t[:], EPS)
        eps_b = const.tile([P, 1], F32, tag="eps_b", name="eps_b")
        nc.vector.memset(eps_b[:], EPS)
        dum = const.tile([1, 1], F32, tag="dum", name="dum")
        nc.vector.memset(dum[:], 1.0)

        def dummy_ln():
            # touch Ln so walrus inserts the nat_log_exp table load HERE,
            # while the PE is busy with mains, not on the LN critical path.
            # Self-chained (and DMA'd out at the end) so DCE keeps it.
            # Ln(0*x + 1) == 0 stays finite for CoreSim's NaN check.
            nc.scalar.activation(dum[:], dum[:], AF.Ln, bias=1.0, scale=0.0)

        def load_w_slabs(pool, dram, n, width, tagpfx, eng=None):
            eng = eng or nc.sync
            ts = []
            for k in range(n):
                t = pool.tile([P, width], F16, tag=f"{tagpfx}{k}",
                              name=f"{tagpfx}{k}")
                eng.dma_start(out=t[:], in_=dram[k * P:(k + 1) * P, :])
                ts.append(t)
            return ts

        def attn_mains(kx_tiles, res_tiles, w_tiles, bias_pb):
            """mains + evac + squares + running DVE tile-sums.
            The 8-tile sums (sxt, sqt) leave only ONE ones-matmul each for
            the partition reduction later, instead of 8 — the partial sums
            ride the idle DVE capacity instead of the PE.
            returns (x_tiles, sxt, sqt)."""
            x_tiles = []
            sq_tiles = []
            sxt = stp.tile([P, CHUNK], F16, tag="sxt", name="sxt")
            sqt = stp.tile([P, CHUNK], F16, tag="sqt", name="sqt")
            for m in range(MT):
                ps = mps.tile([P, CHUNK], F32, tag="ps", name="ps")
                for k in range(KT):
                    nc.tensor.matmul(
                        ps[:], lhsT=w_tiles[k][:, m * P:(m + 1) * P],
                        rhs=kx_tiles[k][:],
                        start=(k == 0), stop=(k == KT - 1))
                xt = xp.tile([P, CHUNK], F16, tag=f"x{m}", name=f"x{m}")
                nc.vector.scalar_tensor_tensor(
                    xt[:], ps[:], bias_pb[:, m:m + 1], res_tiles[m][:],
                    OP.add, OP.add)
                sq = sqp.tile([P, CHUNK], F16, tag="s", name="s")
                nc.scalar.activation(sq[:], xt[:], AF.Square)
                x_tiles.append(xt)
                sq_tiles.append(sq)
                if m == 1:
                    nc.vector.tensor_add(sxt[:], x_tiles[0][:], x_tiles[1][:])
                    nc.vector.tensor_add(sqt[:], sq_tiles[0][:], sq_tiles[1][:])
                elif m > 1:
                    nc.vector.tensor_add(sxt[:], sxt[:], xt[:])
                    nc.vector.tensor_add(sqt[:], sqt[:], sq[:])
            dummy_ln()
            return x_tiles, sxt, sqt

        def ln_tail(x_tiles, sxt, sqt, g_pb):
            """-> y1[m] fp16 = (x - mean) * rstd * g   (beta folded on host).
            Partition reduction AND broadcast in one GPSIMD all-reduce
            (result replicated on all partitions) — no PE involvement.
            rstd = exp(-0.5*ln(var+eps)); Ln/Exp share one ACT table set."""
            ar_x = arp.tile([P, CHUNK], F32, tag="arx", name="arx")
            nc.gpsimd.partition_all_reduce(ar_x[:], sxt[:], P,
                                           bass_isa.ReduceOp.add)
            ar_q = arp.tile([P, CHUNK], F32, tag="arq", name="arq")
            nc.gpsimd.partition_all_reduce(ar_q[:], sqt[:], P,
                                           bass_isa.ReduceOp.add)
            mu = smp.tile([P, CHUNK], F32, tag="mu", name="mu")
            nc.vector.tensor_scalar(mu[:], ar_x[:], 1.0 / E, None, OP.mult)
            m2 = smp.tile([P, CHUNK], F32, tag="m2", name="m2")
            nc.vector.tensor_mul(m2[:], mu[:], mu[:])
            var = smp.tile([P, CHUNK], F32, tag="var", name="var")
            nc.vector.scalar_tensor_tensor(
                var[:], ar_q[:], 1.0 / E, m2[:], OP.mult, OP.subtract)
            nmB = bbp.tile([P, CHUNK], F16, tag="nmB", name="nmB")
            nc.vector.tensor_scalar(nmB[:], mu[:], -1.0, None, OP.mult)
            nc.scalar.activation(var[:], var[:], AF.Ln, bias=eps_b[:])
            rB = bbp.tile([P, CHUNK], F16, tag="rB", name="rB")
            nc.scalar.activation(rB[:], var[:], AF.Exp, scale=-0.5)
            y_tiles = []
            for m in range(MT):
                yt = y1p.tile([P, CHUNK], F16, tag=f"y{m}", name=f"y{m}")
                nc.vector.tensor_add(yt[:], x_tiles[m][:], nmB[:])
                nc.vector.scalar_tensor_tensor(
                    yt[:], yt[:], g_pb[:, m:m + 1], rB[:], OP.mult, OP.mult)
                y_tiles.append(yt)
            return y_tiles

        def ffn(y_tiles, w1_dram, b1_pb, w2_dram, b2_pb, out_dram, cs, opool,
                otag, per_m):
            """y2[m] fp16 = y + W2.T@gelu(W1.T@y + b1') + b2'; streams to
            out_dram[:, cs]."""
            w1_t = load_w_slabs(w1p, w1_dram, KT, H2, "w1_")
            h_tiles = []
            for hm in range(HT):
                ps = mps.tile([P, CHUNK], F32, tag="ps", name="ps")
                for k in range(KT):
                    nc.tensor.matmul(
                        ps[:], lhsT=w1_t[k][:, hm * P:(hm + 1) * P],
                        rhs=y_tiles[k][:],
                        start=(k == 0), stop=(k == KT - 1))
                ht = hp.tile([P, CHUNK], F16, tag=f"h{hm}", name=f"h{hm}")
                nc.scalar.activation(ht[:], ps[:], AF.Gelu,
                                     bias=b1_pb[:, hm:hm + 1])
                h_tiles.append(ht)
            w2_t = load_w_slabs(w2p, w2_dram, HT, E, "w2_")
            y2_tiles = []
            for m in range(MT):
                ps = mps.tile([P, CHUNK], F32, tag="ps", name="ps")
                for k in range(HT):
                    nc.tensor.matmul(
                        ps[:], lhsT=w2_t[k][:, m * P:(m + 1) * P],
                        rhs=h_tiles[k][:],
                        start=(k == 0), stop=(k == HT - 1))
                tg = f"{otag}{m}" if per_m else otag
                yt = opool.tile([P, CHUNK], F16, tag=tg, name=f"{otag}{m}")
                nc.vector.scalar_tensor_tensor(
                    yt[:], ps[:], b2_pb[:, m:m + 1], y_tiles[m][:],
                    OP.add, OP.add)
                nc.sync.dma_start(out=out_dram[m * P:(m + 1) * P, cs],
                                  in_=yt[:])
                y2_tiles.append(yt)
            dummy_ln()
            return y2_tiles

        def load_chunk_inputs(c, vres_eng=None):
            cs = slice(c * CHUNK, (c + 1) * CHUNK)
            vres_eng = vres_eng or nc.sync
            noun_t = []
            vres_t = []
            for k in range(KT):
                t = nounp.tile([P, CHUNK], F16, tag=f"n{k}", name=f"n{k}")
                nc.sync.dma_start(out=t[:], in_=nT[k * P:(k + 1) * P, cs])
                noun_t.append(t)
                t = vresp.tile([P, CHUNK], F16, tag=f"v{k}", name=f"v{k}")
                vres_eng.dma_start(out=t[:], in_=vT[k * P:(k + 1) * P, cs])
                vres_t.append(t)
            return noun_t, vres_t

        import os as _os
        _REP = int(_os.environ.get("BENCH_REPEAT", "1"))
        # chunk-0 critical-path DMAs lead the queue, split across both
        # HWDGE rings (SP + ACT; ACT is idle at t=0); bias vectors follow
        noun_0, vres_0 = load_chunk_inputs(0, vres_eng=nc.scalar)
        wv_0 = load_w_slabs(wvp, wvo1, KT, E, "wv", eng=nc.scalar)
        bvo1_pb = _load_pvec(nc, const, bvo1, MT, "bvo1")
        bvo2_pb = _load_pvec(nc, const, bvo2, MT, "bvo2")
        lnvg_pb = _load_pvec(nc, const, lnvg, MT, "lnvg")
        lnng_pb = _load_pvec(nc, const, lnng, MT, "lnng")
        b1v_pb = _load_pvec(nc, const, b1v, HT, "b1v")
        b2v_pb = _load_pvec(nc, const, b2v, MT, "b2v")
        b1n_pb = _load_pvec(nc, const, b1n, HT, "b1n")
        b2n_pb = _load_pvec(nc, const, b2n, MT, "b2n")
        for _rep in range(_REP):
            if _rep == 0:
                noun_t, vres_t, wv_t = noun_0, vres_0, wv_0
            else:
                noun_t, vres_t = load_chunk_inputs(0)
                wv_t = load_w_slabs(wvp, wvo1, KT, E, "wv")
            A_pend = attn_mains(noun_t, vres_t, wv_t, bvo1_pb)
            A_res = noun_t
            for c in range(NCH):
                cs = slice(c * CHUNK, (c + 1) * CHUNK)
                verb1 = ln_tail(*A_pend, lnvg_pb)
                verb2 = ffn(verb1, w1v, b1v_pb, w2v, b2v_pb, verb_out, cs,
                            vo_p, "vo", True)
                wv_t = load_w_slabs(wvp, wvo2, KT, E, "wv")
                C_pend = attn_mains(verb2, A_res, wv_t, bvo2_pb)
                if c < NCH - 1:
                    noun_t, vres_t = load_chunk_inputs(c + 1)
                    wv_t = load_w_slabs(wvp, wvo1, KT, E, "wv")
                    nxt = attn_mains(noun_t, vres_t, wv_t, bvo1_pb)
                    nxt_res = noun_t
                noun1 = ln_tail(*C_pend, lnng_pb)
                ffn(noun1, w1n, b1n_pb, w2n, b2n_pb, noun_out, cs,
                    no_p, "no", False)
                if c < NCH - 1:
                    A_pend, A_res = nxt, nxt_res
        # keep the dummy-Ln chain live past DCE
        nc.sync.dma_start(out=scratch[:, :], in_=dum[:])

    nc.finalize()
    return nc


_prog_cache = {}


def _get_program():
    if "nc" not in _prog_cache:
        _prog_cache["nc"] = _build_program()
    return _prog_cache["nc"]


def _prepare_maps(inputs):
    f32 = np.float32
    f16 = np.float16
    g = {k: np.asarray(v, f32) for k, v in inputs.items()}

    def fold(p):
        w = g[f"{p}_wo"] @ g[f"{p}_wv"]
        b = g[f"{p}_wo"] @ g[f"{p}_bv"] + g[f"{p}_bo"]
        return np.ascontiguousarray(w.T).astype(f16), np.ascontiguousarray(b)

    wvo1, bvo1 = fold("v2n")
    wvo2, bvo2 = fold("n2v")
    common = {
        "wvo1": wvo1, "bvo1": _pack_pvec(bvo1),
        "wvo2": wvo2, "bvo2": _pack_pvec(bvo2),
        "lnvg": _pack_pvec(g["ln_v_g"]), "lnng": _pack_pvec(g["ln_n_g"]),
        "w1v": np.ascontiguousarray(g["fv_w1"].T).astype(f16),
        "b1v": _pack_pvec(g["fv_b1"] + g["fv_w1"] @ g["ln_v_b"]),
        "w2v": np.ascontiguousarray(g["fv_w2"].T).astype(f16),
        "b2v": _pack_pvec(g["fv_b2"] + g["ln_v_b"]),
        "w1n": np.ascontiguousarray(g["fn_w1"].T).astype(f16),
        "b1n": _pack_pvec(g["fn_b1"] + g["fn_w1"] @ g["ln_n_b"]),
        "w2n": np.ascontiguousarray(g["fn_w2"].T).astype(f16),
        "b2n": _pack_pvec(g["fn_b2"] + g["ln_n_b"]),
    }
    vT = np.ascontiguousarray(g["verb_features"].T).astype(f16)  # [E, 16384]
    nT = np.ascontiguousarray(g["noun_features"].T).astype(f16)
    in_maps = []
    for i in range(NCORES):
        cs = slice(i * B, (i + 1) * B)
        m = dict(common)
        m["vT"] = np.ascontiguousarray(vT[:, cs])
        m["nT"] = np.ascontiguousarray(nT[:, cs])
        m["scratch"] = np.zeros((1, 1), f32)
        in_maps.append(m)
    return in_maps


def kernel(**inputs):
    nc = _get_program()
    in_maps = _prepare_maps(inputs)
    res = run_bass_kernel_spmd(nc, in_maps, list(range(NCORES))).results
    verb = np.concatenate([res[i]["verb_out"] for i in range(NCORES)], axis=1)
    noun = np.concatenate([res[i]["noun_out"] for i in range(NCORES)], axis=1)
    return (np.ascontiguousarray(verb.T).astype(np.float32),
            np.ascontiguousarray(noun.T).astype(np.float32))
